# revision 22
# baseline (speedup 1.0000x reference)
"""Trainium2 Bass kernel for a fused multi-head attention block.

Reference computation (B=2, S=2048, H=1024, NH=16, HD=64):
    qh/kh/vh = (x @ W + b) per head
    energy   = qh @ kh^T  (full S x S per head)
    attn     = softmax(where(mask==0, -1e9, energy) / sqrt(H))
    out      = attn @ vh
    y        = out @ Wfc + bfc + q (residual)
    return LayerNorm(y) * gamma + beta

Sharding: data-parallel over batch (2 groups of 4 cores) x tensor-parallel
over heads (4 heads per core). Wq/Wk/Wv column-sharded, Wfc row-sharded,
ReduceScatter(add) over each 4-core group after fc, then per-core
residual+LayerNorm on its 512-row output slice.

Per-core kernel design (v2):
  * All GEMM inputs are fp8e4m3; weights are pre-scaled by 32 on the host
    (values ~N(0, .02*32) sit in e4m3's normal range) and the PSUM drains
    rescale by 1/32. Projections and fc use DoubleRow perf mode (two
    128-row contraction tiles per instruction at half cycle cost).
  * The attention core stays bf16: q/k projections produce TRANSPOSED
    activations qhT/khT [256, S]; scores are computed transposed
    (energy^T[k, q]); the masked softmax is multiplicative
    P = exp(E/32) * maskT with the mask streamed once per (half, kj) and
    shared by both head-pairs.
  * attn@V accumulates out^T[d|denom, q] in PSUM; the appended vext ones
    column is 1/32 so the fp8 out^T tile can be written as 32*out/denom
    (normal fp8 range); fc's 32*32 gain is removed by a 1/1024 drain.
  * The softmax steady state is ACT(exp)-bound (~2.1us per k-tile pair of
    exps); everything else (mask mul, epilogues, fc drains, LN) is kept
    off ACT so exp never stalls. fc + ReduceScatter + LayerNorm for each
    q-half are emitted as soon as that half's attention finishes so the
    tail overlaps the other half.
"""

import numpy as np
import ml_dtypes

import concourse.bass as bass
import concourse.mybir as mybir
from concourse import bacc, tile
from concourse.bass_utils import run_bass_kernel_spmd

B, S, H, NH = 2, 2048, 1024, 16
HD = H // NH                  # 64
NCORES = 8
TPG = 4                       # cores per tensor-parallel group
HPC = NH // TPG               # 4 heads per core
DC = HPC * HD                 # 256 head-dims per core
SR = S // TPG                 # 512 output rows per core
INV_SCALE = 1.0 / float(H) ** 0.5   # 1/32
WSC = 32.0                    # host-side weight scale (fp8 range)
EPS = 1e-5

FP = mybir.dt.float32
BF = mybir.dt.bfloat16
F8 = mybir.dt.float8e4
F32 = np.float32
BF16 = ml_dtypes.bfloat16
F8E4 = ml_dtypes.float8_e4m3

G = 4                         # DoubleRow contraction groups (2x128 each)
ST = S // 128                 # 16 seq tiles
QC = S // 512                 # 4 q-chunks of 512
RT = SR // 128                # 4 row tiles in the final phase
E1 = HD + 1                   # 65: head dims + denominator column

ts = bass.ts
AF = mybir.ActivationFunctionType
ALU = mybir.AluOpType
DR = mybir.MatmulPerfMode.DoubleRow


def _build_nc():
    nc = bacc.Bacc(
        "TRN2",
        target_bir_lowering=False,
        debug=False,
        num_devices=NCORES,
    )

    # ---- per-core DRAM I/O ----
    # x inputs are fp8, DoubleRow-interleaved: row blocks (256g+128i+p)
    # stored as [4*128, 2*S] with pair slot i at free offset i*S.
    qT = nc.dram_tensor("qT", [G * 128, 2 * S], F8, kind="ExternalInput")
    kTt = nc.dram_tensor("kTt", [G * 128, 2 * S], F8, kind="ExternalInput")
    vT = nc.dram_tensor("vT", [G * 128, 2 * S], F8, kind="ExternalInput")
    maskT = nc.dram_tensor("maskT", [S, S], BF, kind="ExternalInput")
    wq = nc.dram_tensor("wq", [G * 128, 2 * DC], F8, kind="ExternalInput")
    wk = nc.dram_tensor("wk", [G * 128, 2 * DC], F8, kind="ExternalInput")
    wv = nc.dram_tensor("wv", [G * 128, 2 * DC], F8, kind="ExternalInput")
    wfc = nc.dram_tensor("wfc", [128, 2 * H], F8, kind="ExternalInput")
    bq = nc.dram_tensor("bq", [DC, 1], FP, kind="ExternalInput")
    bk = nc.dram_tensor("bk", [DC, 1], FP, kind="ExternalInput")
    bv = nc.dram_tensor("bv", [1, DC], FP, kind="ExternalInput")
    resid = nc.dram_tensor("resid", [SR, H], FP, kind="ExternalInput")
    gamma = nc.dram_tensor("gamma", [1, H], FP, kind="ExternalInput")
    beta = nc.dram_tensor("beta", [1, H], FP, kind="ExternalInput")
    out = nc.dram_tensor("out", [SR, H], FP, kind="ExternalOutput")

    with tile.TileContext(nc) as tc:
        with (
            tc.tile_pool(name="const", bufs=1) as cpool,
            tc.tile_pool(name="stream", bufs=2) as spool,
            tc.tile_pool(name="mask", bufs=17) as mpool,
            tc.tile_pool(name="work", bufs=4) as wpool,
            tc.tile_pool(name="epi", bufs=2) as epool,
            tc.tile_pool(name="fin", bufs=2) as fpool,
            tc.tile_pool(name="psum", bufs=1, space="PSUM") as ppool,
            tc.tile_pool(name="psA", bufs=2, space="PSUM") as ppoolA,
            tc.tile_pool(name="dram", bufs=1, space="DRAM") as dpool,
            tc.tile_pool(name="dram2", bufs=2, space="DRAM") as dpool2,
        ):
            # ---------- q/k projections (transposed outputs [DC, S]) ----------
            # DoubleRow: lhsT = w[g] viewed [128, 2, m], rhs = x[g] viewed
            # [128, 2, n]; four g-groups accumulate the full 1024 contraction.
            bias_sb = {}
            for name, dram in (("bq", bq), ("bk", bk)):
                tiles = []
                for nt in range(2):
                    t = cpool.tile([128, 1], FP, tag=f"{name}{nt}")
                    nc.sync.dma_start(out=t[:], in_=dram[ts(nt, 128), :])
                    tiles.append(t)
                bias_sb[name] = tiles

            qhT_sb = [
                cpool.tile([128, S], BF, tag=f"qhT{nt}", name=f"qhT{nt}")
                for nt in range(2)
            ]
            khT_sb = [
                cpool.tile([128, S], BF, tag=f"khT{nt}", name=f"khT{nt}")
                for nt in range(2)
            ]
            for bname, wdram, xdram, outsb in (
                ("bq", wq, qT, qhT_sb),
                ("bk", wk, kTt, khT_sb),
            ):
                w_tiles, x_tiles = [], []
                for g in range(G):
                    wt = spool.tile([128, 2 * DC], F8, tag=f"w{g}")
                    nc.sync.dma_start(out=wt[:], in_=wdram[ts(g, 128), :])
                    w_tiles.append(wt.rearrange("p (i m) -> p i m", i=2))
                # x loads go through the Pool/SWDGE path: its descriptor
                # generation is off the (serial) HWDGE dispatch unit, which
                # is reserved for the latency-critical mask stream
                for g in range(G):
                    xt = spool.tile([128, 2 * S], F8, tag=f"x{g}")
                    nc.sync.dma_start(out=xt[:], in_=xdram[ts(g, 128), :])
                    x_tiles.append(xt.rearrange("p (i n) -> p i n", i=2))
                for qc in range(QC):
                    for nt in range(2):
                        ps = ppoolA.tile([128, 512], FP, tag="A")
                        for g in range(G):
                            nc.tensor.matmul(
                                ps[:],
                                lhsT=w_tiles[g][:, :, ts(nt, 128)],
                                rhs=x_tiles[g][:, :, ts(qc, 512)],
                                start=(g == 0),
                                stop=(g == G - 1),
                                perf_mode=DR,
                            )
                        nc.scalar.activation(
                            outsb[nt][:, ts(qc, 512)],
                            ps[:],
                            AF.Identity,
                            scale=1.0 / WSC,
                            bias=bias_sb[bname][nt][:],
                        )

            # ---------- v projection inputs (compute is interleaved into the
            # first attention iterations so exp starts ~15us earlier) ----------
            bvB = cpool.tile([128, DC], FP, tag="bvB")
            nc.sync.dma_start(out=bvB[:], in_=bv[:].broadcast_to([128, DC]))
            wv_tiles, xv_tiles = [], []
            for g in range(G):
                wt = spool.tile([128, 2 * DC], F8, tag=f"w{g}")
                nc.sync.dma_start(out=wt[:], in_=wv[ts(g, 128), :])
                wv_tiles.append(wt.rearrange("p (i m) -> p i m", i=2))
            for g in range(G):
                xt = spool.tile([128, 2 * S], F8, tag=f"x{g}")
                nc.gpsimd.dma_start(out=xt[:], in_=vT[ts(g, 128), :])
                xv_tiles.append(xt.rearrange("p (i n) -> p i n", i=2))
            vext_sb = []

            def emit_vproj_st(st):
                vx = cpool.tile([128, HPC * E1], BF, tag=f"vext{st}")
                ps = ppoolA.tile([128, DC], FP, tag="A")
                for g in range(G):
                    nc.tensor.matmul(
                        ps[:],
                        lhsT=xv_tiles[g][:, :, ts(st, 128)],
                        rhs=wv_tiles[g][:, :, :],
                        start=(g == 0),
                        stop=(g == G - 1),
                        perf_mode=DR,
                    )
                # denominator column is 1/32 so the fp8 out^T can carry 32/denom
                for h in range(HPC):
                    nc.vector.memset(vx[:, h * E1 + HD : h * E1 + E1], 1.0 / WSC)
                v3 = vx.rearrange("p (h e) -> p h e", e=E1)[:, :, 0:HD]
                p3 = ps.rearrange("p (h e) -> p h e", e=HD)
                b3 = bvB.rearrange("p (h e) -> p h e", e=HD)
                nc.vector.scalar_tensor_tensor(
                    v3, p3, 1.0 / WSC, b3, ALU.mult, ALU.add
                )
                vext_sb.append(vx)

            # fc weights + LN constants: consumed mid/late; emitted lazily so
            # their DMAs queue behind the attention-critical loads.
            wfc_sb = cpool.tile([128, 2 * H], F8, tag="wfc", name="wfc")
            wfc3 = wfc_sb.rearrange("p (i h) -> p i h", i=2)
            gammaB = cpool.tile([128, H], FP, tag="gammaB")
            betaB = cpool.tile([128, H], FP, tag="betaB")
            late_loads = [False]

            def emit_late_loads():
                if late_loads[0]:
                    return
                late_loads[0] = True
                nc.sync.dma_start(out=wfc_sb[:], in_=wfc[:])
                nc.sync.dma_start(
                    out=gammaB[:], in_=gamma[:].broadcast_to([128, H])
                )
                nc.sync.dma_start(out=betaB[:], in_=beta[:].broadcast_to([128, H]))

            # out^T, fp8, value 32*out/denom: [128 dims, dg slot, S]
            outT2 = cpool.tile([128, 2 * S], F8, tag="outT2", name="outT2")
            outT3 = outT2.rearrange("p (i s) -> p i s", i=2)

            # ---------- attention ----------
            LAG = 3
            B_tiles = {}
            pending = []

            ones64 = cpool.tile([1, 64], BF, tag="ones64")
            nc.vector.memset(ones64[:], 1.0)

            def emit_epilogue(hp, half, hh, Bt, psum_rb):
                q0 = 1024 * half
                # copy B out of PSUM first: releases the accumulator bank for
                # the next block before the reciprocal/broadcast chain runs
                Bc = epool.tile([E1, 1024], FP, tag="Bc", name="Bc", bufs=3)
                nc.scalar.activation(Bc[:], Bt[:], AF.Identity)
                if psum_rb:
                    # attention is over: broadcast 1/denom across partitions
                    # with a rank-1 matmul into a free PSUM slot (no DMA hops)
                    rc = epool.tile([1, 1024], BF, tag="rc", name="rc")
                    with nc.allow_low_precision(
                        reason="1/denom feeds a bf16 matmul broadcast"
                    ):
                        nc.vector.reciprocal(rc[:], Bc[64:65, :])
                    rb = ppoolA.tile([64, 1024], FP, tag="A", name="rbps")
                    for c in range(2):
                        nc.tensor.matmul(
                            rb[:, ts(c, 512)],
                            lhsT=ones64[:],
                            rhs=rc[0:1, ts(c, 512)],
                            start=True,
                            stop=True,
                        )
                else:
                    rcf = epool.tile([1, 1024], FP, tag="rcf", name="rcf")
                    nc.vector.reciprocal(rcf[:], Bc[64:65, :])
                    rdram = dpool2.tile([1, 1024], FP, tag="rdram", name="rdram")
                    nc.sync.dma_start(out=rdram[:], in_=rcf[:])
                    rb = epool.tile([64, 1024], FP, tag="rb", name="rb")
                    nc.sync.dma_start(
                        out=rb[:], in_=rdram[:].broadcast_to([64, 1024])
                    )
                if hh == 0:
                    nc.vector.tensor_mul(
                        outT2[0:64, hp * S + q0 : hp * S + q0 + 1024],
                        Bc[0:64, :],
                        rb[:],
                    )
                else:
                    osc = epool.tile([64, 1024], F8, tag="osc", name="osc")
                    nc.vector.tensor_mul(osc[:], Bc[0:64, :], rb[:])
                    nc.gpsimd.dma_start(
                        out=outT2[64:128, hp * S + q0 : hp * S + q0 + 1024],
                        in_=osc[:],
                    )

            def emit_attnv(ent, psum_rb=False):
                hp, half, kj, hh, Pm = ent
                h = 2 * hp + hh
                Bt = B_tiles[(hp, half)][hh]
                for c in range(2):
                    nc.tensor.matmul(
                        Bt[:, ts(c, 512)],
                        lhsT=vext_sb[kj][:, h * E1 : (h + 1) * E1],
                        rhs=Pm[:, ts(c, 512)],
                        start=(kj == 0),
                        stop=(kj == ST - 1),
                    )
                if kj == ST - 1:
                    emit_epilogue(hp, half, hh, Bt, psum_rb)

            # y_part chunk i covers s-rows [512i, 512(i+1)); after each chunk's
            # four s-tiles finish, a ReduceScatter over the 4-core group hands
            # this core rows [512i+128r, 512i+128(r+1)) (r = group rank).
            y_chunks = [
                dpool.tile([SR, H], BF, tag=f"y_part{i}", name=f"y_part{i}")
                for i in range(RT)
            ]
            z_chunks = [
                dpool.tile([128, H], BF, tag=f"z{i}", name=f"z{i}")
                for i in range(RT)
            ]

            def emit_fc_st(st, drain_act):
                ps = ppoolA.tile([128, H], FP, tag="A")
                for hc in range(2):
                    nc.tensor.matmul(
                        ps[:, ts(hc, 512)],
                        lhsT=outT3[:, :, ts(st, 128)],
                        rhs=wfc3[:, :, ts(hc, 512)],
                        start=True,
                        stop=True,
                        perf_mode=DR,
                    )
                yb = fpool.tile([128, H], BF, tag="yb", bufs=3)
                if drain_act:
                    nc.scalar.activation(
                        yb[:], ps[:], AF.Identity, scale=1.0 / (WSC * WSC)
                    )
                else:
                    nc.vector.tensor_scalar_mul(yb[:], ps[:], 1.0 / (WSC * WSC))
                nc.sync.dma_start(
                    out=y_chunks[st // 4][ts(st % 4, 128), :], in_=yb[:]
                )
                if st % 4 == 3:
                    nc.gpsimd.collective_compute(
                        "ReduceScatter",
                        ALU.add,
                        replica_groups=[[0, 1, 2, 3], [4, 5, 6, 7]],
                        ins=[y_chunks[st // 4][:]],
                        outs=[z_chunks[st // 4][:]],
                    )

            def emit_ln_rt(rt, eng):
                # eng picks who runs the big row ops: "dve" mid-attention
                # (so exp on ACT never waits), "pool"/"dve" at the tail so
                # the two final layernorms run on different engines.
                e = nc.gpsimd if eng == "pool" else nc.vector
                zbf = fpool.tile([128, H], BF, tag="zbf", bufs=2)
                nc.sync.dma_start(out=zbf[:], in_=z_chunks[rt][:])
                rs = fpool.tile([128, H], FP, tag="rs")
                nc.sync.dma_start(out=rs[:], in_=resid[ts(rt, 128), :])
                musum = fpool.tile([128, 1], FP, tag="musum")
                zt = fpool.tile([128, H], FP, tag="zt")
                e.scalar_tensor_tensor(
                    zt[:], zbf[:], 0.0, rs[:], ALU.add, ALU.add, accum_out=musum[:]
                )
                nmu = fpool.tile([128, 1], FP, tag="nmu")
                nc.vector.tensor_scalar_mul(nmu[:], musum[:], -1.0 / H)
                if eng == "act":
                    nc.scalar.activation(zt[:], zt[:], AF.Identity, bias=nmu[:])
                else:
                    e.tensor_scalar_add(zt[:], zt[:], nmu[:])
                ssq = fpool.tile([128, 1], FP, tag="ssq")
                e.scalar_tensor_tensor(
                    rs[:], zt[:], 0.0, zt[:], ALU.add, ALU.mult, accum_out=ssq[:]
                )
                varp = fpool.tile([128, 1], FP, tag="varp")
                nc.vector.tensor_scalar(
                    varp[:], ssq[:], 1.0 / H, EPS, ALU.mult, ALU.add
                )
                sdev = fpool.tile([128, 1], FP, tag="sdev")
                nc.scalar.activation(sdev[:], varp[:], AF.Sqrt)
                rstd = fpool.tile([128, 1], FP, tag="rstd")
                nc.vector.reciprocal(rstd[:], sdev[:])
                e.scalar_tensor_tensor(
                    rs[:], zt[:], rstd[:], gammaB[:], ALU.mult, ALU.mult
                )
                ot = fpool.tile([128, H], FP, tag="ot")
                e.tensor_add(ot[:], rs[:], betaB[:])
                nc.sync.dma_start(out=out[ts(rt, 128), :], in_=ot[:])

            for half in range(2):
                q0 = 1024 * half
                masks = {}
                for hp in range(2):
                    B_tiles[(hp, half)] = [
                        ppool.tile(
                            [E1, 1024], FP, tag=f"attB{hh}",
                            name=f"attB{hh}_{hp}_{half}",
                        )
                        for hh in range(2)
                    ]
                    for kj in range(ST):
                        # interleaved off-critical work, paced one slot per
                        # k-tile so the PE/DVE insertions stay under the
                        # ~2.1us of exp the ACT queue holds per k-tile; fc is
                        # deferred 6 k-tiles past the half boundary so the
                        # prior half's epilogue chains complete before fc
                        # heads the in-order PE queue:
                        if hp == 0:
                            mt = mpool.tile([128, 1024], BF, tag="mask", name="mask")
                            nc.sync.dma_start(
                                out=mt[:], in_=maskT[ts(kj, 128), q0 : q0 + 1024]
                            )
                            masks[kj] = mt
                        mt = masks[kj]
                        if half == 0 and hp == 0:
                            # v projection; vext[kj] is needed LAG iters later
                            emit_vproj_st(kj)
                        if half == 1:
                            if hp == 0 and kj == 0:
                                emit_late_loads()
                            slot = 16 * hp + kj
                            if 6 <= slot < 22 and (slot - 6) % 2 == 0:
                                emit_fc_st((slot - 6) // 2, drain_act=False)
                            elif slot == 23:
                                emit_ln_rt(0, eng="dve")
                            elif slot == 26:
                                emit_ln_rt(1, eng="dve")
                        for hh in range(2):
                            hb = 64 * hh
                            A = ppoolA.tile([128, 1024], FP, tag="A", name="A")
                            for c in range(2):
                                nc.tensor.matmul(
                                    A[:, ts(c, 512)],
                                    lhsT=khT_sb[hp][hb : hb + 64, ts(kj, 128)],
                                    rhs=qhT_sb[hp][hb : hb + 64, q0 + 512 * c : q0 + 512 * (c + 1)],
                                    start=True,
                                    stop=True,
                                )
                            P = wpool.tile([128, 1024], BF, tag="P", name="P", bufs=5)
                            nc.scalar.activation(P[:], A[:], AF.Exp, scale=INV_SCALE)
                            Pm = wpool.tile(
                                [128, 1024], BF, tag="Pm", name="Pm", bufs=7
                            )
                            nc.vector.tensor_mul(Pm[:], P[:], mt[:])
                            pending.append((hp, half, kj, hh, Pm))
                            if len(pending) > LAG:
                                emit_attnv(pending.pop(0))
                    if hp == 1:
                        # flush at the half boundary so this half's epilogues
                        # emit before the next phase enters the engine queues
                        for ent in pending:
                            emit_attnv(ent, psum_rb=(half == 1))
                        pending = []
                # the second half's fc + reduce-scatter + layernorm have
                # nothing left to hide under: run them with drains balanced
                # across the now-idle ACT and DVE
                if half == 1:
                    for st in range(8, 16):
                        emit_fc_st(st, drain_act=(st % 2 == 0))
                    emit_ln_rt(2, eng="pool")
                    emit_ln_rt(3, eng="dve")

    nc.compile()
    return nc


_NC_CACHE = {}


def _get_nc():
    if "nc" not in _NC_CACHE:
        _NC_CACHE["nc"] = _build_nc()
    return _NC_CACHE["nc"]


def _dr_fold(a):
    """[1024, N] -> DoubleRow-interleaved [4*128, 2*N] fp8 (rows 256g+128i+p
    at block (g*128+p, i*N))."""
    n = a.shape[1]
    return np.ascontiguousarray(
        a.reshape(G, 2, 128, n).transpose(0, 2, 1, 3).reshape(G * 128, 2 * n)
    ).astype(F8E4)


def _prep_inputs(q, k, v, mask, Wq, bq, Wk, bk, Wv, bv, Wfc, bfc, gamma, beta):
    """Build the 8 per-core input maps on the host (sharding + layout)."""
    q = np.asarray(q, F32)
    k = np.asarray(k, F32)
    v = np.asarray(v, F32)
    mask = np.asarray(mask)
    in_maps = []
    qT_b, kT_b, vT_b, maskT_b = [], [], [], []
    for b in range(B):
        qT_b.append(_dr_fold(np.ascontiguousarray(q[b].T)))
        kT_b.append(_dr_fold(np.ascontiguousarray(k[b].T)))
        vT_b.append(_dr_fold(np.ascontiguousarray(v[b].T)))
        maskT_b.append(np.ascontiguousarray(mask[b, 0].T).astype(BF16))
    Wq_f, Wk_f, Wv_f, Wfc_f = (
        np.asarray(w, F32) * WSC for w in (Wq, Wk, Wv, Wfc)
    )
    for c in range(NCORES):
        b, g = c // TPG, c % TPG
        cols = slice(g * DC, (g + 1) * DC)
        wfc_sl = Wfc_f[cols, :]  # [256, 1024]
        wfc_dr = np.ascontiguousarray(
            wfc_sl.reshape(2, 128, H).transpose(1, 0, 2).reshape(128, 2 * H)
        ).astype(F8E4)
        in_maps.append({
            "qT": qT_b[b],
            "kTt": kT_b[b],
            "vT": vT_b[b],
            "maskT": maskT_b[b],
            "wq": _dr_fold(Wq_f[:, cols]),
            "wk": _dr_fold(Wk_f[:, cols]),
            "wv": _dr_fold(Wv_f[:, cols]),
            "wfc": wfc_dr,
            "bq": np.asarray(bq, F32)[cols].reshape(DC, 1),
            "bk": np.asarray(bk, F32)[cols].reshape(DC, 1),
            "bv": np.asarray(bv, F32)[cols].reshape(1, DC),
            "resid": np.ascontiguousarray(
                np.concatenate(
                    [
                        q[b, 512 * i + 128 * g : 512 * i + 128 * (g + 1)]
                        for i in range(RT)
                    ]
                )
                + np.asarray(bfc, F32)[None, :]
            ),
            "gamma": np.asarray(gamma, F32).reshape(1, H),
            "beta": np.asarray(beta, F32).reshape(1, H),
        })
    return in_maps


_LAST_RUN_S = [None]


def kernel(q, k, v, mask, Wq, bq, Wk, bk, Wv, bv, Wfc, bfc, gamma, beta):
    import time

    nc = _get_nc()
    in_maps = _prep_inputs(
        q, k, v, mask, Wq, bq, Wk, bk, Wv, bv, Wfc, bfc, gamma, beta
    )
    t0 = time.perf_counter()
    res = run_bass_kernel_spmd(nc, in_maps, list(range(NCORES)))
    _LAST_RUN_S[0] = time.perf_counter() - t0
    full = np.empty((B, S, H), F32)
    for c in range(NCORES):
        b, r = c // TPG, c % TPG
        o = res.results[c]["out"]
        for i in range(RT):
            full[b, 512 * i + 128 * r : 512 * i + 128 * (r + 1)] = o[
                128 * i : 128 * (i + 1)
            ]
    return full


# revision 30
# speedup vs baseline: 1.3458x; 1.3458x over previous
"""Trainium2 Bass kernel for a fused multi-head attention block.

Reference computation (B=2, S=2048, H=1024, NH=16, HD=64):
    qh/kh/vh = (x @ W + b) per head
    energy   = qh @ kh^T  (full S x S per head)
    attn     = softmax(where(mask==0, -1e9, energy) / sqrt(H))
    out      = attn @ vh
    y        = out @ Wfc + bfc + q (residual)
    return LayerNorm(y) * gamma + beta

Sharding: data-parallel over batch (2 groups of 4 cores) x tensor-parallel
over heads (4 heads per core). Wq/Wk/Wv column-sharded, Wfc row-sharded,
ReduceScatter(add) over each 4-core group after fc, then per-core
residual+LayerNorm on its 512-row output slice.

Per-core kernel design (v2):
  * q/k/v/weights are fp8e4m3; weights pre-scaled by 32 on the host
    (values ~N(0, .02*32) sit in e4m3's normal range) and the PSUM drains
    rescale by 1/32. Projections and fc use DoubleRow perf mode (two
    128-row contraction tiles per instruction at half cycle cost); their
    inputs are host-interleaved into the [K, 2, N] pair layout.
  * The attention core stays bf16 (the fp8 noise enters only through the
    projections, whose output error averages down over the K=1024
    contraction): q/k projections produce TRANSPOSED activations
    qhT/khT [256, S]; scores are computed transposed (energy^T[k, q]);
    the masked softmax is multiplicative P = exp(E/32) * maskT, with
    each mask tile streamed once per (half, kj) and shared by both
    head-pairs.
  * attn@V accumulates out^T[d|denom, q] in PSUM; the appended vext
    "ones" column is 1/32 so the fp8 out^T tile carries 32*out/denom
    (normal fp8 range); fc's 32*32 gain is removed by a 1/1024 drain.
  * The softmax steady state is ACT(exp)-bound (~2.1us per k-tile pair
    of exps, ~133us floor); everything else is kept off the ACT queue so
    exp never waits: mask muls + epilogues on DVE, v-projection compute
    interleaved one s-tile per k-tile into the first attention block,
    the previous half's fc / ReduceScatter / LayerNorm interleaved into
    the next half's iterations (fc deferred 6 k-tiles so the epilogue
    chains land before fc heads the in-order PE queue).
  * B accumulators are copied out of PSUM (ACT) before the reciprocal /
    broadcast chain so the next block's attn@V gets its bank back
    immediately; the final epilogues broadcast 1/denom with a rank-1 PE
    matmul instead of the DRAM round-trip to shorten the tail.
  * DMA dispatch (single serial HWDGE unit, ~0.6us each) is budgeted:
    merged single-dispatch weight loads, large input tiles, and the v
    input routed through the Pool/SWDGE path so the per-k-tile mask
    stream owns the HWDGE queue.
"""

import numpy as np
import ml_dtypes

import concourse.bass as bass
import concourse.mybir as mybir
from concourse import bacc, tile
from concourse.bass_utils import run_bass_kernel_spmd

B, S, H, NH = 2, 2048, 1024, 16
HD = H // NH                  # 64
NCORES = 8
TPG = 4                       # cores per tensor-parallel group
HPC = NH // TPG               # 4 heads per core
DC = HPC * HD                 # 256 head-dims per core
SR = S // TPG                 # 512 output rows per core
INV_SCALE = 1.0 / float(H) ** 0.5   # 1/32
WSC = 32.0                    # host-side weight scale (fp8 range)
EPS = 1e-5

FP = mybir.dt.float32
BF = mybir.dt.bfloat16
F8 = mybir.dt.float8e4
F32 = np.float32
BF16 = ml_dtypes.bfloat16
F8E4 = ml_dtypes.float8_e4m3

G = 4                         # DoubleRow contraction groups (2x128 each)
ST = S // 128                 # 16 seq tiles
QC = S // 512                 # 4 q-chunks of 512
RT = SR // 128                # 4 row tiles in the final phase
E1 = HD + 1                   # 65: head dims + denominator column

ts = bass.ts
AF = mybir.ActivationFunctionType
ALU = mybir.AluOpType
DR = mybir.MatmulPerfMode.DoubleRow


def _build_nc():
    nc = bacc.Bacc(
        "TRN2",
        target_bir_lowering=False,
        debug=False,
        num_devices=NCORES,
    )

    # ---- per-core DRAM I/O ----
    # x inputs are fp8, DoubleRow-interleaved: row blocks (256g+128i+p)
    # stored as [4*128, 2*S] with pair slot i at free offset i*S.
    qT = nc.dram_tensor("qT", [G * 128, 2 * S], F8, kind="ExternalInput")
    kTt = nc.dram_tensor("kTt", [G * 128, 2 * S], F8, kind="ExternalInput")
    vT = nc.dram_tensor("vT", [G * 128, 2 * S], F8, kind="ExternalInput")
    maskT = nc.dram_tensor("maskT", [S, S], BF, kind="ExternalInput")
    wq = nc.dram_tensor("wq", [G * 128, 2 * DC], F8, kind="ExternalInput")
    wk = nc.dram_tensor("wk", [G * 128, 2 * DC], F8, kind="ExternalInput")
    wv = nc.dram_tensor("wv", [G * 128, 2 * DC], F8, kind="ExternalInput")
    wfc = nc.dram_tensor("wfc", [128, 2 * H], F8, kind="ExternalInput")
    bq = nc.dram_tensor("bq", [DC, 1], FP, kind="ExternalInput")
    bk = nc.dram_tensor("bk", [DC, 1], FP, kind="ExternalInput")
    bv = nc.dram_tensor("bv", [1, DC], FP, kind="ExternalInput")
    resid = nc.dram_tensor("resid", [SR, H], FP, kind="ExternalInput")
    gamma = nc.dram_tensor("gamma", [1, H], FP, kind="ExternalInput")
    beta = nc.dram_tensor("beta", [1, H], FP, kind="ExternalInput")
    out = nc.dram_tensor("out", [SR, H], FP, kind="ExternalOutput")

    with tile.TileContext(nc) as tc:
        with (
            tc.tile_pool(name="const", bufs=1) as cpool,
            tc.tile_pool(name="stream", bufs=2) as spool,
            tc.tile_pool(name="mask", bufs=17) as mpool,
            tc.tile_pool(name="work", bufs=4) as wpool,
            tc.tile_pool(name="epi", bufs=2) as epool,
            tc.tile_pool(name="fin", bufs=2) as fpool,
            tc.tile_pool(name="psum", bufs=1, space="PSUM") as ppool,
            tc.tile_pool(name="psA", bufs=2, space="PSUM") as ppoolA,
            tc.tile_pool(name="dram", bufs=1, space="DRAM") as dpool,
            tc.tile_pool(name="dram2", bufs=2, space="DRAM") as dpool2,
        ):
            # ---------- q/k projections (transposed outputs [DC, S]) ----------
            # DoubleRow: lhsT = w[g] viewed [128, 2, m], rhs = x[g] viewed
            # [128, 2, n]; four g-groups accumulate the full 1024 contraction.
            bias_names = (("bq", bq), ("bk", bk))
            bias_sb = {
                name: [
                    cpool.tile(
                        [128, 1], FP, tag=f"{name}{nt}", name=f"{name}{nt}"
                    )
                    for nt in range(2)
                ]
                for name, _ in bias_names
            }

            def load_w_merged(wdram):
                # all four DoubleRow g-groups in one dispatch: the serial
                # HWDGE unit makes per-DMA dispatch (~0.6us) precious
                wt = spool.tile([128, G * 2 * DC], F8, tag="w")
                nc.sync.dma_start(
                    out=wt[:],
                    in_=wdram[:, :].rearrange("(g p) m -> p g m", p=128),
                )
                w4 = wt.rearrange("p (g i m) -> p g i m", g=G, i=2)
                return [w4[:, g] for g in range(G)]

            qhT_sb = [
                cpool.tile([128, S], BF, tag=f"qhT{nt}", name=f"qhT{nt}")
                for nt in range(2)
            ]
            khT_sb = [
                cpool.tile([128, S], BF, tag=f"khT{nt}", name=f"khT{nt}")
                for nt in range(2)
            ]
            for bname, wdram, xdram, outsb in (
                ("bq", wq, qT, qhT_sb),
                ("bk", wk, kTt, khT_sb),
            ):
                w_tiles = load_w_merged(wdram)
                x_tiles = []
                for g in range(G):
                    xt = spool.tile([128, 2 * S], F8, tag=f"x{g}")
                    nc.sync.dma_start(out=xt[:], in_=xdram[ts(g, 128), :])
                    x_tiles.append(xt.rearrange("p (i n) -> p i n", i=2))
                # bias loads dispatched behind x: first consumer is the qc0
                # drain, well after the x transfers
                for nt in range(2):
                    nc.sync.dma_start(
                        out=bias_sb[bname][nt][:],
                        in_=dict(bias_names)[bname][ts(nt, 128), :],
                    )
                for qc in range(QC):
                    for nt in range(2):
                        ps = ppoolA.tile([128, 512], FP, tag="A")
                        for g in range(G):
                            nc.tensor.matmul(
                                ps[:],
                                lhsT=w_tiles[g][:, :, ts(nt, 128)],
                                rhs=x_tiles[g][:, :, ts(qc, 512)],
                                start=(g == 0),
                                stop=(g == G - 1),
                                perf_mode=DR,
                            )
                        nc.scalar.activation(
                            outsb[nt][:, ts(qc, 512)],
                            ps[:],
                            AF.Identity,
                            scale=1.0 / WSC,
                            bias=bias_sb[bname][nt][:],
                        )

            # ---------- v projection inputs (compute is interleaved into the
            # first attention iterations so exp starts ~15us earlier) ----------
            bvB = cpool.tile([128, DC], FP, tag="bvB")
            nc.sync.dma_start(out=bvB[:], in_=bv[:].broadcast_to([128, DC]))
            wv_tiles = load_w_merged(wv)
            xv_tiles = []
            for g in range(G):
                xt = spool.tile([128, 2 * S], F8, tag=f"x{g}")
                nc.gpsimd.dma_start(out=xt[:], in_=vT[ts(g, 128), :])
                xv_tiles.append(xt.rearrange("p (i n) -> p i n", i=2))
            vext_sb = []

            def emit_vproj_st(st):
                vx = cpool.tile([128, HPC * E1], BF, tag=f"vext{st}")
                ps = ppoolA.tile([128, DC], FP, tag="A")
                for g in range(G):
                    nc.tensor.matmul(
                        ps[:],
                        lhsT=xv_tiles[g][:, :, ts(st, 128)],
                        rhs=wv_tiles[g][:, :, :],
                        start=(g == 0),
                        stop=(g == G - 1),
                        perf_mode=DR,
                    )
                # denominator column is 1/32 so the fp8 out^T can carry 32/denom
                for h in range(HPC):
                    nc.vector.memset(vx[:, h * E1 + HD : h * E1 + E1], 1.0 / WSC)
                v3 = vx.rearrange("p (h e) -> p h e", e=E1)[:, :, 0:HD]
                p3 = ps.rearrange("p (h e) -> p h e", e=HD)
                b3 = bvB.rearrange("p (h e) -> p h e", e=HD)
                nc.vector.scalar_tensor_tensor(
                    v3, p3, 1.0 / WSC, b3, ALU.mult, ALU.add
                )
                vext_sb.append(vx)

            # fc weights + LN constants: consumed mid/late; emitted lazily so
            # their DMAs queue behind the attention-critical loads.
            wfc_sb = cpool.tile([128, 2 * H], F8, tag="wfc", name="wfc")
            wfc3 = wfc_sb.rearrange("p (i h) -> p i h", i=2)
            gammaB = cpool.tile([128, H], FP, tag="gammaB")
            betaB = cpool.tile([128, H], FP, tag="betaB")
            late_loads = [False]

            def emit_late_loads():
                if late_loads[0]:
                    return
                late_loads[0] = True
                nc.sync.dma_start(out=wfc_sb[:], in_=wfc[:])
                nc.sync.dma_start(
                    out=gammaB[:], in_=gamma[:].broadcast_to([128, H])
                )
                nc.sync.dma_start(out=betaB[:], in_=beta[:].broadcast_to([128, H]))

            # out^T, fp8, value 32*out/denom: [128 dims, dg slot, S]
            outT2 = cpool.tile([128, 2 * S], F8, tag="outT2", name="outT2")
            outT3 = outT2.rearrange("p (i s) -> p i s", i=2)

            # ---------- attention ----------
            LAG = 3
            B_tiles = {}
            pending = []

            ones64 = cpool.tile([1, 64], BF, tag="ones64")
            nc.vector.memset(ones64[:], 1.0)

            def emit_epilogue(hp, half, hh, Bt, psum_rb):
                q0 = 1024 * half
                # copy B out of PSUM first: releases the accumulator bank for
                # the next block before the reciprocal/broadcast chain runs
                Bc = epool.tile([E1, 1024], FP, tag="Bc", name="Bc", bufs=3)
                nc.scalar.activation(Bc[:], Bt[:], AF.Identity)
                if psum_rb:
                    # attention is over: broadcast 1/denom across partitions
                    # with a rank-1 matmul into a free PSUM slot (no DMA hops)
                    rc = epool.tile([1, 1024], BF, tag="rc", name="rc")
                    with nc.allow_low_precision(
                        reason="1/denom feeds a bf16 matmul broadcast"
                    ):
                        nc.vector.reciprocal(rc[:], Bc[64:65, :])
                    rb = ppoolA.tile([64, 1024], FP, tag="A", name="rbps")
                    for c in range(2):
                        nc.tensor.matmul(
                            rb[:, ts(c, 512)],
                            lhsT=ones64[:],
                            rhs=rc[0:1, ts(c, 512)],
                            start=True,
                            stop=True,
                        )
                else:
                    rcf = epool.tile([1, 1024], FP, tag="rcf", name="rcf")
                    nc.vector.reciprocal(rcf[:], Bc[64:65, :])
                    rdram = dpool2.tile([1, 1024], FP, tag="rdram", name="rdram")
                    nc.sync.dma_start(out=rdram[:], in_=rcf[:])
                    rb = epool.tile([64, 1024], FP, tag="rb", name="rb")
                    nc.sync.dma_start(
                        out=rb[:], in_=rdram[:].broadcast_to([64, 1024])
                    )
                if hh == 0:
                    nc.vector.tensor_mul(
                        outT2[0:64, hp * S + q0 : hp * S + q0 + 1024],
                        Bc[0:64, :],
                        rb[:],
                    )
                else:
                    osc = epool.tile([64, 1024], F8, tag="osc", name="osc")
                    nc.vector.tensor_mul(osc[:], Bc[0:64, :], rb[:])
                    nc.sync.dma_start(
                        out=outT2[64:128, hp * S + q0 : hp * S + q0 + 1024],
                        in_=osc[:],
                    )

            def emit_attnv(ent, psum_rb=False):
                hp, half, kj, hh, Pm = ent
                h = 2 * hp + hh
                Bt = B_tiles[(hp, half)][hh]
                for c in range(2):
                    nc.tensor.matmul(
                        Bt[:, ts(c, 512)],
                        lhsT=vext_sb[kj][:, h * E1 : (h + 1) * E1],
                        rhs=Pm[:, ts(c, 512)],
                        start=(kj == 0),
                        stop=(kj == ST - 1),
                    )
                if kj == ST - 1:
                    emit_epilogue(hp, half, hh, Bt, psum_rb)

            # y_part chunk i covers s-rows [512i, 512(i+1)); after each chunk's
            # four s-tiles finish, a ReduceScatter over the 4-core group hands
            # this core rows [512i+128r, 512i+128(r+1)) (r = group rank).
            y_chunks = [
                dpool.tile([SR, H], BF, tag=f"y_part{i}", name=f"y_part{i}")
                for i in range(RT)
            ]
            z_chunks = [
                dpool.tile([128, H], BF, tag=f"z{i}", name=f"z{i}")
                for i in range(RT)
            ]

            def emit_fc_st(st, drain_act):
                ps = ppoolA.tile([128, H], FP, tag="A")
                for hc in range(2):
                    nc.tensor.matmul(
                        ps[:, ts(hc, 512)],
                        lhsT=outT3[:, :, ts(st, 128)],
                        rhs=wfc3[:, :, ts(hc, 512)],
                        start=True,
                        stop=True,
                        perf_mode=DR,
                    )
                yb = fpool.tile([128, H], BF, tag="yb", bufs=3)
                if drain_act:
                    nc.scalar.activation(
                        yb[:], ps[:], AF.Identity, scale=1.0 / (WSC * WSC)
                    )
                else:
                    nc.vector.tensor_scalar_mul(yb[:], ps[:], 1.0 / (WSC * WSC))
                nc.sync.dma_start(
                    out=y_chunks[st // 4][ts(st % 4, 128), :], in_=yb[:]
                )
                if st % 4 == 3:
                    nc.gpsimd.collective_compute(
                        "ReduceScatter",
                        ALU.add,
                        replica_groups=[[0, 1, 2, 3], [4, 5, 6, 7]],
                        ins=[y_chunks[st // 4][:]],
                        outs=[z_chunks[st // 4][:]],
                    )

            def emit_ln_rt(rt, eng):
                # eng picks who runs the big row ops: "dve" mid-attention
                # (so exp on ACT never waits), "pool"/"dve" at the tail so
                # the two final layernorms run on different engines.
                e = nc.gpsimd if eng == "pool" else nc.vector
                zbf = fpool.tile([128, H], BF, tag="zbf", bufs=2)
                nc.sync.dma_start(out=zbf[:], in_=z_chunks[rt][:])
                rs = fpool.tile([128, H], FP, tag="rs")
                nc.sync.dma_start(out=rs[:], in_=resid[ts(rt, 128), :])
                musum = fpool.tile([128, 1], FP, tag="musum")
                zt = fpool.tile([128, H], FP, tag="zt")
                e.scalar_tensor_tensor(
                    zt[:], zbf[:], 0.0, rs[:], ALU.add, ALU.add, accum_out=musum[:]
                )
                nmu = fpool.tile([128, 1], FP, tag="nmu")
                nc.vector.tensor_scalar_mul(nmu[:], musum[:], -1.0 / H)
                if eng == "act":
                    nc.scalar.activation(zt[:], zt[:], AF.Identity, bias=nmu[:])
                else:
                    e.tensor_scalar_add(zt[:], zt[:], nmu[:])
                ssq = fpool.tile([128, 1], FP, tag="ssq")
                e.scalar_tensor_tensor(
                    rs[:], zt[:], 0.0, zt[:], ALU.add, ALU.mult, accum_out=ssq[:]
                )
                varp = fpool.tile([128, 1], FP, tag="varp")
                nc.vector.tensor_scalar(
                    varp[:], ssq[:], 1.0 / H, EPS, ALU.mult, ALU.add
                )
                sdev = fpool.tile([128, 1], FP, tag="sdev")
                nc.scalar.activation(sdev[:], varp[:], AF.Sqrt)
                rstd = fpool.tile([128, 1], FP, tag="rstd")
                nc.vector.reciprocal(rstd[:], sdev[:])
                e.scalar_tensor_tensor(
                    rs[:], zt[:], rstd[:], gammaB[:], ALU.mult, ALU.mult
                )
                ot = fpool.tile([128, H], FP, tag="ot")
                e.tensor_add(ot[:], rs[:], betaB[:])
                nc.sync.dma_start(out=out[ts(rt, 128), :], in_=ot[:])

            for half in range(2):
                q0 = 1024 * half
                masks = {}
                for hp in range(2):
                    B_tiles[(hp, half)] = [
                        ppool.tile(
                            [E1, 1024], FP, tag=f"attB{hh}",
                            name=f"attB{hh}_{hp}_{half}",
                        )
                        for hh in range(2)
                    ]
                    for kj in range(ST):
                        # interleaved off-critical work, paced one slot per
                        # k-tile so the PE/DVE insertions stay under the
                        # ~2.1us of exp the ACT queue holds per k-tile; fc is
                        # deferred 6 k-tiles past the half boundary so the
                        # prior half's epilogue chains complete before fc
                        # heads the in-order PE queue:
                        if hp == 0:
                            mt = mpool.tile([128, 1024], BF, tag="mask", name="mask")
                            nc.sync.dma_start(
                                out=mt[:], in_=maskT[ts(kj, 128), q0 : q0 + 1024]
                            )
                            masks[kj] = mt
                        mt = masks[kj]
                        if half == 0 and hp == 0:
                            # v projection; vext[kj] is needed LAG iters later
                            emit_vproj_st(kj)
                        if half == 1:
                            if hp == 0 and kj == 0:
                                emit_late_loads()
                            slot = 16 * hp + kj
                            if 6 <= slot < 22 and (slot - 6) % 2 == 0:
                                emit_fc_st((slot - 6) // 2, drain_act=False)
                            elif slot == 23:
                                emit_ln_rt(0, eng="dve")
                            elif slot == 26:
                                emit_ln_rt(1, eng="dve")
                        for hh in range(2):
                            hb = 64 * hh
                            A = ppoolA.tile([128, 1024], FP, tag="A", name="A")
                            for c in range(2):
                                nc.tensor.matmul(
                                    A[:, ts(c, 512)],
                                    lhsT=khT_sb[hp][hb : hb + 64, ts(kj, 128)],
                                    rhs=qhT_sb[hp][hb : hb + 64, q0 + 512 * c : q0 + 512 * (c + 1)],
                                    start=True,
                                    stop=True,
                                )
                            P = wpool.tile([128, 1024], BF, tag="P", name="P", bufs=5)
                            nc.scalar.activation(P[:], A[:], AF.Exp, scale=INV_SCALE)
                            Pm = wpool.tile(
                                [128, 1024], BF, tag="Pm", name="Pm", bufs=7
                            )
                            nc.vector.tensor_mul(Pm[:], P[:], mt[:])
                            pending.append((hp, half, kj, hh, Pm))
                            if len(pending) > LAG:
                                emit_attnv(pending.pop(0))
                    if hp == 1:
                        # flush at the half boundary so this half's epilogues
                        # emit before the next phase enters the engine queues;
                        pending.sort(key=lambda e: (e[2], -e[3]))
                        for ent in pending:
                            emit_attnv(ent, psum_rb=(half == 1))
                        pending = []
                # the second half's fc + reduce-scatter + layernorm have
                # nothing left to hide under: run them with drains balanced
                # across the now-idle ACT and DVE
                if half == 1:
                    for st in range(8, 16):
                        emit_fc_st(st, drain_act=(st % 2 == 0))
                    emit_ln_rt(2, eng="act")
                    emit_ln_rt(3, eng="act")

    nc.compile()
    return nc


_NC_CACHE = {}


def _get_nc():
    if "nc" not in _NC_CACHE:
        _NC_CACHE["nc"] = _build_nc()
    return _NC_CACHE["nc"]


def _dr_fold(a):
    """[1024, N] -> DoubleRow-interleaved [4*128, 2*N] fp8 (rows 256g+128i+p
    at block (g*128+p, i*N))."""
    n = a.shape[1]
    return np.ascontiguousarray(
        a.reshape(G, 2, 128, n).transpose(0, 2, 1, 3).reshape(G * 128, 2 * n)
    ).astype(F8E4)


def _prep_inputs(q, k, v, mask, Wq, bq, Wk, bk, Wv, bv, Wfc, bfc, gamma, beta):
    """Build the 8 per-core input maps on the host (sharding + layout)."""
    q = np.asarray(q, F32)
    k = np.asarray(k, F32)
    v = np.asarray(v, F32)
    mask = np.asarray(mask)
    in_maps = []
    qT_b, kT_b, vT_b, maskT_b = [], [], [], []
    for b in range(B):
        qT_b.append(_dr_fold(np.ascontiguousarray(q[b].T)))
        kT_b.append(_dr_fold(np.ascontiguousarray(k[b].T)))
        vT_b.append(_dr_fold(np.ascontiguousarray(v[b].T)))
        maskT_b.append(np.ascontiguousarray(mask[b, 0].T).astype(BF16))
    Wq_f, Wk_f, Wv_f, Wfc_f = (
        np.asarray(w, F32) * WSC for w in (Wq, Wk, Wv, Wfc)
    )
    for c in range(NCORES):
        b, g = c // TPG, c % TPG
        cols = slice(g * DC, (g + 1) * DC)
        wfc_sl = Wfc_f[cols, :]  # [256, 1024]
        wfc_dr = np.ascontiguousarray(
            wfc_sl.reshape(2, 128, H).transpose(1, 0, 2).reshape(128, 2 * H)
        ).astype(F8E4)
        in_maps.append({
            "qT": qT_b[b],
            "kTt": kT_b[b],
            "vT": vT_b[b],
            "maskT": maskT_b[b],
            "wq": _dr_fold(Wq_f[:, cols]),
            "wk": _dr_fold(Wk_f[:, cols]),
            "wv": _dr_fold(Wv_f[:, cols]),
            "wfc": wfc_dr,
            "bq": np.asarray(bq, F32)[cols].reshape(DC, 1),
            "bk": np.asarray(bk, F32)[cols].reshape(DC, 1),
            "bv": np.asarray(bv, F32)[cols].reshape(1, DC),
            "resid": np.ascontiguousarray(
                np.concatenate(
                    [
                        q[b, 512 * i + 128 * g : 512 * i + 128 * (g + 1)]
                        for i in range(RT)
                    ]
                )
                + np.asarray(bfc, F32)[None, :]
            ),
            "gamma": np.asarray(gamma, F32).reshape(1, H),
            "beta": np.asarray(beta, F32).reshape(1, H),
        })
    return in_maps


_LAST_RUN_S = [None]


def kernel(q, k, v, mask, Wq, bq, Wk, bk, Wv, bv, Wfc, bfc, gamma, beta):
    import time

    nc = _get_nc()
    in_maps = _prep_inputs(
        q, k, v, mask, Wq, bq, Wk, bk, Wv, bv, Wfc, bfc, gamma, beta
    )
    t0 = time.perf_counter()
    res = run_bass_kernel_spmd(nc, in_maps, list(range(NCORES)))
    _LAST_RUN_S[0] = time.perf_counter() - t0
    full = np.empty((B, S, H), F32)
    for c in range(NCORES):
        b, r = c // TPG, c % TPG
        o = res.results[c]["out"]
        for i in range(RT):
            full[b, 512 * i + 128 * r : 512 * i + 128 * (r + 1)] = o[
                128 * i : 128 * (i + 1)
            ]
    return full
